# revision 1
# baseline (speedup 1.0000x reference)
"""GAT-style attention layer on 8 TRN2 NeuronCores (raw Bass, SPMD).

Math (per batch element b, N=256 nodes, F=64 feats, HID=128):
  x      = leaky_relu(src @ W_lin^T, 0.2)                  [N, HID]
  d      = x @ a_dst                                       [N]
  sq_ij  = ||src_i - src_j||^2  (Gram trick)               [N, N]
  e_ij   = d_j + coef * sqrt(sq_ij * adj_ij),  coef = W_edge . a_edge
  out    = softmax_j(e_ij)          (mask is all-ones; adj diag zeroed)

The s_i = x@a_src term of the reference cancels in softmax_j (constant
shift along the softmax axis) and is not computed at all.

Sharding: data-parallel over batch B=8 -> one batch element per core.

Device kernel per core (raw Bass engine programs; walrus build allows
only ONE sync wait per compute instruction -> standalone wait_ge). The
whole attention matrix is computed TRANSPOSED (pt[j, i]) so that d_j
becomes a per-partition ACT bias; sq is symmetric so the same Gram
matmuls serve, and the host sends adj transposed:
  - ALL matmuls in fp16 (single PE pass): two sq halves (K=66 with
    rank-1 rsq/ones rows; lhsT = [srcT;ones;rsq] pairs with host-built
    rhs2 = [-2srcT;rsq;ones] in one DMA buffer), x^T = wlt^T srcT, and
    two [128,1] matmuls that produce d as a PSUM COLUMN per half.
  - sqT * adjT is computed BEFORE the sqrt (diag of adj is host-zeroed;
    DVE multiplies fp32 PSUM by the uint8 adj directly), so fp16 matmul
    noise on the ~0 diagonal can never reach ln of a negative number;
    ln(0 + 1e-6 bias) is finite and the result is crushed by softmax.
  - sqrt as exp(0.5*ln(x)): one ACT table set covers both; the table is
    pre-warmed with a dummy activation during the input DMA. The dist
    exp folds in |coef| via a ln|coef| bias; the softmax exp folds in
    sign(coef) via scale and (d_j - 26) via a per-partition bias.
  - the device ships only the softmax NUMERATOR exp(logits - 26) as
    fp16 (max logit ~33 -> exp(~7) fits fp16); the host transposes back
    and normalizes. No on-device row sums, reciprocals, or normalize.
  - no DMA completion wait at the end: the output lands during the
    multi-us Block-exit teardown, long before the host reads it.
  - DMA bytes: megaP fp16 85KB (sync queue) + adjT uint8 with a_dst
    fp16 embedded as 2 byte-columns, 65KB (scalar queue, parallel).
The mask input is all-ones in this problem; the device kernel relies on
that (verified on host, with a numpy fallback if it ever isn't). The
host also falls back to numpy if the device result is non-finite
(fp16 exp overflow would need logits > 36.7; this problem's are ~33).
"""

import math
from contextlib import ExitStack

import numpy as np

import concourse.bass as bass
from concourse import mybir
from concourse.bass_utils import run_bass_kernel_spmd

B, N, F_IN, HID = 8, 256, 64, 128
NEG_SLOPE = 0.2
F16 = mybir.dt.float16
F32 = mybir.dt.float32
U8 = mybir.dt.uint8
AF = mybir.ActivationFunctionType
ALU = mybir.AluOpType

K = F_IN + 2  # 66
WP = N + HID  # 384: srcT|ones|rsq cols 0:256, wlt cols 256:384
WPD = WP + N  # 640: + rhs2 = [-2srcT; rsq; ones] in cols 384:640
WB = 2 * N + 2  # 514: adjT half0 | adjT half1 | a_dst fp16 bytes

_NC_CACHE: dict = {}


def _build_nc(coef: float) -> bass.Bass:
    nc = bass.Bass()

    megaP = nc.declare_dram_parameter("megaP", [K, WPD], F16, isOutput=False)
    adjq = nc.declare_dram_parameter("adjq", [128, WB], U8, isOutput=False)
    out = nc.declare_dram_parameter("out", [HID, 2 * N], F16, isOutput=True)

    ctx = ExitStack()
    with ctx:
        sb = lambda shape, dt, name: ctx.enter_context(nc.sbuf_tensor(name, shape, dt))
        psum = lambda shape, name: ctx.enter_context(nc.psum_tensor(name, shape, F32))
        sem = lambda name: ctx.enter_context(nc.semaphore(name))

        megaP_sb = sb([K, WPD], F16, "megaP_sb")
        adj_sb = sb([128, WB], U8, "adj_sb")
        xt_sb = sb([HID, N], F16, "xt_sb")
        relu08 = sb([HID, N], F32, "relu08")
        sqadj = sb([128, 2 * N], F16, "sqadj")
        ln_sb = sb([128, 2 * N], F32, "ln_sb")
        dist = sb([128, 2 * N], F32, "dist")
        pt_sb = sb([128, 2 * N], F16, "pt_sb")
        dbias = sb([128, 2], F32, "dbias")
        warm = sb([128, 1], F32, "warm")
        eps = sb([128, 1], F32, "eps")
        lncoef = sb([128, 1], F32, "lncoef")

        xt_ps = psum([HID, N], "xt_ps")
        sq_ps0 = psum([128, N], "sq_ps0")
        sq_ps1 = psum([128, N], "sq_ps1")
        dcol_ps = psum([128, 2], "dcol_ps")

        qP = sem("qP")
        qB = sem("qB")
        qOut = sem("qOut")
        sPE = sem("sPE")
        sG = sem("sG")
        sV = sem("sV")
        sA = sem("sA")

        adst = adj_sb[:, 2 * N : WB].bitcast(F16)  # [128, 1]
        sgn = 1.0 if coef > 0 else -1.0

        with nc.Block(no_gpsimd_drain=True) as block:

            @block.sync
            def _(sync):
                sync.dma_start(megaP_sb[:], megaP[:]).then_inc(qP, 16)
                sync.wait_ge(sA, 5)  # exp half 0
                sync.dma_start(out[:, 0:N], pt_sb[:, 0:N]).then_inc(qOut, 16)
                sync.wait_ge(sA, 7)  # exp half 1
                # no completion wait: the output lands during the multi-us
                # Block-exit drain/teardown, long before the host reads it
                sync.dma_start(out[:, N : 2 * N], pt_sb[:, N : 2 * N]).then_inc(
                    qOut, 16
                )



            @block.gpsimd
            def _(gpsimd):
                gpsimd.memset(eps[:], 1.0e-6).then_inc(sG, 1)  # 1
                gpsimd.memset(lncoef[:], float(math.log(abs(coef)))).then_inc(
                    sG, 1
                )  # 2


            @block.tensor
            def _(tensor):
                tensor.wait_ge(qP, 16)
                tensor.matmul(
                    sq_ps0[:], megaP_sb[:, 0:128], megaP_sb[:, WP:WPD],
                    start=True, stop=True,
                ).then_inc(sPE, 1)  # 1
                tensor.matmul(
                    xt_ps[:],
                    megaP_sb[0:F_IN, N : N + HID],
                    megaP_sb[0:F_IN, 0:N],
                    start=True,
                    stop=True,
                ).then_inc(sPE, 1)  # 2
                tensor.matmul(
                    sq_ps1[:], megaP_sb[:, 128:256], megaP_sb[:, WP:WPD],
                    start=True, stop=True,
                ).then_inc(sPE, 1)  # 3
                # d as a COLUMN (per-partition): dcol[p] = a_dst . x_p
                tensor.wait_ge(qB, 16)
                tensor.wait_ge(sV, 4)  # xt_sb
                tensor.matmul(
                    dcol_ps[:, 0:1], xt_sb[:, 0:128], adst, start=True, stop=True
                ).then_inc(sPE, 1)  # 4
                tensor.matmul(
                    dcol_ps[:, 1:2], xt_sb[:, 128:256], adst, start=True, stop=True
                ).then_inc(sPE, 1)  # 5

            @block.vector
            def _(vector):
                vector.memset(warm[:], 1.0).then_inc(sV, 1)  # 1
                # sqadjT = sq * adjT BEFORE the sqrt (sq is symmetric; adj is
                # host-transposed): multiplies straight from PSUM by uint8
                vector.wait_ge(sPE, 1)
                vector.wait_ge(qB, 16)
                vector.tensor_mul(sqadj[:, 0:N], sq_ps0[:], adj_sb[:, 0:N]).then_inc(
                    sV, 1
                )  # 2
                # leaky_relu(x) = 0.2*x + 0.8*relu(x), one PSUM read per op
                vector.wait_ge(sPE, 2)
                vector.tensor_scalar(
                    relu08[:], xt_ps[:], 0.0, 1.0 - NEG_SLOPE, op0=ALU.max, op1=ALU.mult
                ).then_inc(sV, 1)  # 3
                vector.wait_ge(sV, 3)
                vector.scalar_tensor_tensor(
                    xt_sb[:], xt_ps[:], NEG_SLOPE, relu08[:], op0=ALU.mult, op1=ALU.add
                ).then_inc(sV, 1)  # 4
                vector.wait_ge(sPE, 3)
                vector.tensor_mul(
                    sqadj[:, N : 2 * N], sq_ps1[:], adj_sb[:, N : 2 * N]
                ).then_inc(sV, 1)  # 5
                # dbias[p, h] = d[128h + p] - 26 (the -26 keeps exp in fp16)
                vector.wait_ge(sPE, 5)
                vector.tensor_scalar_add(dbias[:], dcol_ps[:], -26.0).then_inc(
                    sV, 1
                )  # 6

            @block.scalar
            def _(scalar):
                # adj (+ embedded a_dst) on the ACT engine's HWDGE ring (its
                # enqueue overlaps the sync queue's megaP transfer)
                scalar.dma_start(adj_sb[:], adjq[:]).then_inc(qB, 16)
                # warm the ln/exp table set while the input DMAs run
                scalar.wait_ge(sV, 1)
                scalar.activation(warm[:], warm[:], AF.Ln).then_inc(sA, 1)  # 1
                scalar.wait_ge(sV, 2)
                scalar.wait_ge(sG, 2)  # eps + lncoef memsets
                scalar.activation(
                    ln_sb[:, 0:N], sqadj[:, 0:N], AF.Ln, bias=eps[:]
                ).then_inc(sA, 1)  # 2
                # dist' = |coef| * sqrt(sqadjT) = exp(0.5*ln + ln|coef|)
                scalar.wait_ge(sA, 2)  # same-engine RAW
                scalar.activation(
                    dist[:, 0:N], ln_sb[:, 0:N], AF.Exp, scale=0.5, bias=lncoef[:]
                ).then_inc(sA, 1)  # 3
                scalar.wait_ge(sV, 5)
                scalar.activation(
                    ln_sb[:, N : 2 * N], sqadj[:, N : 2 * N], AF.Ln, bias=eps[:]
                ).then_inc(sA, 1)  # 4
                # softmax numerator, transposed: pt_jh = exp(sgn*dist' + d_j - 26)
                # (host divides by row sums after transposing back)
                scalar.wait_ge(sV, 6)  # dbias
                scalar.wait_ge(sA, 3)  # same-engine RAW (dist half 0)
                scalar.activation(
                    pt_sb[:, 0:N], dist[:, 0:N], AF.Exp,
                    scale=float(sgn), bias=dbias[:, 0:1],
                ).then_inc(sA, 1)  # 5
                scalar.wait_ge(sA, 4)  # same-engine RAW (ln half 1)
                scalar.activation(
                    dist[:, N : 2 * N], ln_sb[:, N : 2 * N], AF.Exp,
                    scale=0.5, bias=lncoef[:],
                ).then_inc(sA, 1)  # 6
                scalar.wait_ge(sA, 6)  # same-engine RAW
                scalar.activation(
                    pt_sb[:, N : 2 * N], dist[:, N : 2 * N], AF.Exp,
                    scale=float(sgn), bias=dbias[:, 1:2],
                ).then_inc(sA, 1)  # 7

    return nc


def _numpy_reference(src, adj, mask, W_lin, a_src, a_dst, W_edge, a_edge):
    x = np.einsum("bnf,hf->bnh", src, W_lin)
    x = np.where(x > 0, x, NEG_SLOPE * x)
    s = x @ a_src
    d = x @ a_dst
    e = s + np.swapaxes(d, 1, 2)
    coef = float(W_edge[:, 0] @ a_edge[:, 0])
    diff = src[:, :, None, :] - src[:, None, :, :]
    sq = np.sum(diff * diff, axis=-1)
    dist = np.sqrt(np.maximum(sq, 0.0))
    e = e + coef * dist * adj.astype(np.float32)
    a = e * mask.astype(np.float32)
    a = a - a.max(axis=-1, keepdims=True)
    p = np.exp(a)
    return (p / p.sum(axis=-1, keepdims=True)).astype(np.float32)


def _prep_in_maps(src, adj, W_lin, a_dst):
    wlt16 = W_lin.T.astype(np.float16)  # [64, 128]
    adst16 = a_dst.astype(np.float16).reshape(HID)  # [128]
    adst_bytes = adst16.view(np.uint8).reshape(HID, 2)
    in_maps = []
    for b in range(B):
        s16 = src[b].T.astype(np.float16)  # [64, 256]
        rsq = np.sum(s16.astype(np.float32) ** 2, axis=0).astype(np.float16)
        megaP = np.zeros((K, WPD), np.float16)
        megaP[0:F_IN, 0:N] = s16
        megaP[64, 0:N] = np.float16(1.0)
        megaP[65, 0:N] = rsq
        megaP[0:F_IN, N : N + HID] = wlt16
        megaP[0:F_IN, WP:WPD] = np.float16(-2.0) * s16
        megaP[64, WP:WPD] = rsq
        megaP[65, WP:WPD] = np.float16(1.0)
        adjb = adj[b].astype(np.uint8)
        np.fill_diagonal(adjb, 0)  # diagonal never contributes (dist_ii = 0)
        adjbT = np.ascontiguousarray(adjb.T)  # device works transposed
        adjq = np.empty((128, WB), np.uint8)
        adjq[:, 0:N] = adjbT[0:128, :]
        adjq[:, N : 2 * N] = adjbT[128:256, :]
        adjq[:, 2 * N : WB] = adst_bytes
        in_maps.append({"megaP": megaP, "adjq": adjq})
    return in_maps


def kernel(src, adj, mask, W_lin, a_src, a_dst, W_edge, a_edge):
    src = np.asarray(src, dtype=np.float32)
    adj = np.ascontiguousarray(np.asarray(adj, dtype=np.int32))
    W_lin = np.asarray(W_lin, dtype=np.float32)
    a_dst = np.asarray(a_dst, dtype=np.float32)

    if not np.all(np.asarray(mask) == 1):
        return _numpy_reference(
            src, adj, np.asarray(mask), W_lin, np.asarray(a_src, dtype=np.float32),
            a_dst, np.asarray(W_edge, dtype=np.float32),
            np.asarray(a_edge, dtype=np.float32),
        )

    coef = float(np.asarray(W_edge)[:, 0] @ np.asarray(a_edge)[:, 0])
    if coef == 0.0:
        return _numpy_reference(
            src, adj, np.asarray(mask), W_lin, np.asarray(a_src, dtype=np.float32),
            a_dst, np.asarray(W_edge, dtype=np.float32),
            np.asarray(a_edge, dtype=np.float32),
        )

    key = round(coef, 12)
    if key not in _NC_CACHE:
        _NC_CACHE.clear()
        _NC_CACHE[key] = _build_nc(coef)
    nc = _NC_CACHE[key]

    in_maps = _prep_in_maps(src, adj, W_lin, a_dst)
    res = run_bass_kernel_spmd(nc, in_maps, core_ids=list(range(B)))
    result = np.stack(
        [_finish(res.results[b]["out"]) for b in range(B)], axis=0
    )
    if not np.isfinite(result).all():
        # fp16 numerator overflowed (logits > ~36.7) -- not expected for
        # this problem's data, but never return NaN
        return _numpy_reference(
            src, adj, np.asarray(mask), W_lin, np.asarray(a_src, dtype=np.float32),
            a_dst, np.asarray(W_edge, dtype=np.float32),
            np.asarray(a_edge, dtype=np.float32),
        )
    return result


def _finish(pt):
    # pt[j, i-blocks] = exp(logits_ij - 26) fp16, transposed halves side by
    # side; transpose back and normalize on host
    p = np.asarray(pt, np.float32)
    pT = np.concatenate([p[:, 0:N], p[:, N : 2 * N]], axis=0)  # [256j, 256i]
    q = pT.T  # [i, j]
    return q / q.sum(axis=-1, keepdims=True)

